# revision 16
# baseline (speedup 1.0000x reference)
"""Masked attention kernel for Trainium2 (Bass/Tile), 8-core data-parallel.

Full problem: B=16, S=2048, D=64.
  scores = Q @ K^T / 8, masked (mask==0 -> -1e9), p_attn = softmax(scores),
  out = p_attn @ V.  Returns (out, p_attn).

Sharding: batch across 8 cores (2 batches/core), no collectives.

Per-core per-batch plan (float32r matmuls: 1 PE cycle/row at N=512):
  setup:  load Q,K,V natural [128, .]; PE-transpose into 4 column-part tiles
          qt_p/kt_p [65, 512] (row 64: ones / maskbias=(mask-1)*8e9).  The /8
          scale is folded into ACT exp (scale=0.125), the mask into the
          65-row contraction.  V gets a ones column -> vn [128, 16*65].
  pass1:  per q-tile: S[q,k] psum [128,1024]x2 <- qt_tile.T @ kt, E =
          exp(0.125 S) with accum_out -> masked denominators, DVE recip +
          scale -> p_attn tile -> 1MB DMA.
  pass2:  two q-halves; per (half, k-tile): S^T[k,qh] psum [128,1024]
          <- kt_tile.T @ qt (masked, 2 MMs), E^T = exp(0.125 S^T),
          PV: acc[65,1024] += Vext_t.T @ E^T (pipelined one k-tile behind).
          acc row 64 = denominators D[q] for the out normalization.
  final:  acc halves -> accs sbuf; 16 PE transposes [65,128]->[128,65];
          r=1/D (DVE); out chunk = cols 0:64 * r -> one 0.5MB DMA.

Schedule: ACT is the bottleneck (128 exps ~134us) and p_attn DMA second
(~101us).  Per batch the streams are interleaved in trios [1 pass1 tile :
2 pass2 units] so ACT stays dense while the DMA stream never backs up;
batch 1's setup transpose groups are spread into batch 0's trios.
PSUM: "mm" [128,1024] x3 bufs (6 banks) + "acc" [65,1024] x1 (2 banks).
"""

import os
import sys

import numpy as np

for _p in ("/opt/trn_rl_repo", "/root/.axon_site/_ro/trn_rl_repo"):
    if os.path.isdir(_p) and _p not in sys.path:
        sys.path.insert(0, _p)

import concourse.bass as bass
import concourse.mybir as mybir
import concourse.tile as tile
from concourse import bacc
from concourse.bass_utils import run_bass_kernel_spmd
from concourse.masks import make_identity
from contextlib import ExitStack

F32 = mybir.dt.float32
F32R = mybir.dt.float32r
I32 = mybir.dt.int32
AF = mybir.ActivationFunctionType

N_CORES = 8
B_FULL, S, D = 16, 2048, 64
B_LOC = B_FULL // N_CORES  # 2
NT = S // 128  # 16 tiles per seq dim
NP = 4  # column parts of the transposed operands
PW = S // NP  # 512 part width
MASK_SCALE = -8.0e9  # *0.125 -> -1e9
DV = D + 1  # V extended with ones column


def _build():
    nc = bacc.Bacc(
        "TRN2",
        target_bir_lowering=False,
        debug=False,
        enable_asserts=False,
        num_devices=N_CORES,
    )

    q_d = nc.dram_tensor("query", [B_LOC, S, D], F32, kind="ExternalInput").ap()
    k_d = nc.dram_tensor("key", [B_LOC, S, D], F32, kind="ExternalInput").ap()
    v_d = nc.dram_tensor("value", [B_LOC, S, D], F32, kind="ExternalInput").ap()
    m_d = nc.dram_tensor("mask", [B_LOC, 1, S], I32, kind="ExternalInput").ap()
    out_d = nc.dram_tensor("out", [B_LOC, S, D], F32, kind="ExternalOutput").ap()
    p_d = nc.dram_tensor("p_attn", [B_LOC, S, S], F32, kind="ExternalOutput").ap()

    with ExitStack() as ctx:
        tc = ctx.enter_context(tile.TileContext(nc))
        const = ctx.enter_context(tc.tile_pool(name="const", bufs=1))
        nat = ctx.enter_context(tc.tile_pool(name="nat", bufs=1))
        tp = ctx.enter_context(tc.tile_pool(name="tp", bufs=1))
        ep = ctx.enter_context(tc.tile_pool(name="ep", bufs=3))
        sp = ctx.enter_context(tc.tile_pool(name="sp", bufs=1))
        psum = ctx.enter_context(tc.tile_pool(name="psum", bufs=3, space="PSUM"))
        pacc = ctx.enter_context(tc.tile_pool(name="pacc", bufs=1, space="PSUM"))

        ident = const.tile([128, 128], F32, name="ident")
        make_identity(nc, ident[:])
        ones16 = const.tile([128, NT], F32, name="ones16")
        nc.gpsimd.memset(ones16[:], 1.0)

        qts = [[None] * NP for _ in range(B_LOC)]
        kts = [[None] * NP for _ in range(B_LOC)]
        vns = [None] * B_LOC
        accss = [None] * B_LOC

        def setup_tiles(b):
            """Generator: emits loads, then yields after each transpose group."""
            mrow = sp.tile([1, S], F32, tag=f"mrow{b}", name=f"mrow{b}")
            nc.gpsimd.dma_start(mrow[:], m_d[b, 0])

            qn = nat.tile([128, NT * D], F32, tag=f"qn{b}", name=f"qn{b}")
            kn = nat.tile([128, NT * D], F32, tag=f"kn{b}", name=f"kn{b}")
            qv = qn[:].rearrange("p (t d) -> p t d", d=D)
            kv = kn[:].rearrange("p (t d) -> p t d", d=D)
            qs = q_d[b].rearrange("(t p) d -> p t d", p=128)
            ks = k_d[b].rearrange("(t p) d -> p t d", p=128)
            for p in range(NP):
                tsl = slice(4 * p, 4 * p + 4)
                nc.sync.dma_start(qv[:, tsl], qs[:, tsl])
                nc.sync.dma_start(kv[:, tsl], ks[:, tsl])
            vn_raw = nat.tile([128, NT * D], F32, tag=f"vr{b}", name=f"vr{b}")
            nc.sync.dma_start(
                vn_raw[:].rearrange("p (t d) -> p t d", d=D),
                v_d[b].rearrange("(t p) d -> p t d", p=128),
            )
            vn = nat.tile([128, NT * DV], F32R, tag=f"vn{b}", name=f"vn{b}")
            nc.vector.tensor_copy(
                vn[:].rearrange("p (t e) -> p t e", e=DV)[:, :, 0:D],
                vn_raw[:].rearrange("p (t d) -> p t d", d=D),
            )
            nc.vector.tensor_copy(
                vn[:].rearrange("p (t e) -> p t e", e=DV)[:, :, D : D + 1],
                ones16[:].rearrange("p (t u) -> p t u", u=1),
            )
            vns[b] = vn

            def tr_group(src, dst, p):
                tr = psum.tile([D, PW], F32, tag="mm", bufs=3, name="tr")
                for i in range(4):
                    t = 4 * p + i
                    nc.tensor.transpose(
                        tr[:, i * 128 : (i + 1) * 128],
                        src[:, t * D : (t + 1) * D],
                        ident[:],
                    )
                nc.vector.tensor_copy(dst[0:D, :], tr[:])

            def mk_q(p):
                qt = tp.tile([DV, PW], F32R, tag=f"qt{b}{p}", name=f"qt{b}{p}")
                tr_group(qn, qt, p)
                nc.vector.tensor_copy(
                    qt[D : D + 1, :], ones16[0:1, 0:1].broadcast_to([1, PW])
                )
                qts[b][p] = qt

            def mk_k(p):
                kt = tp.tile([DV, PW], F32R, tag=f"kt{b}{p}", name=f"kt{b}{p}")
                tr_group(kn, kt, p)
                nc.vector.tensor_scalar(
                    out=kt[D : D + 1, :],
                    in0=mrow[:, p * PW : (p + 1) * PW],
                    scalar1=1.0,
                    scalar2=-MASK_SCALE,
                    op0=mybir.AluOpType.subtract,
                    op1=mybir.AluOpType.mult,
                )
                kts[b][p] = kt

            # first pass-1 tile needs qt0 + kt0,kt1; first pass-2 unit
            # needs kt0 + qt0,qt1
            for fn, p in (
                (mk_q, 0),
                (mk_k, 0),
                (mk_k, 1),
                (mk_q, 1),
                (mk_q, 2),
                (mk_q, 3),
                (mk_k, 2),
                (mk_k, 3),
            ):
                fn(p)
                yield

        def pass1_tiles(b):
            """Generator: yields after each q-tile unit."""
            qt_p, kt_p = qts[b], kts[b]
            for c in range(NT):
                lhs = qt_p[c // 4][:, (c % 4) * 128 : (c % 4 + 1) * 128]
                e1 = ep.tile([128, S], F32, tag="e1", bufs=6, name="e1")
                dh = sp.tile([128, 2], F32, tag="dh", bufs=6, name="dh")
                for h in range(2):
                    s1 = psum.tile([128, 1024], F32, tag="mm", bufs=3, name="s1")
                    for j in range(2):
                        cj = 2 * h + j
                        nc.tensor.matmul(
                            s1[:, j * 512 : (j + 1) * 512],
                            lhs,
                            kt_p[cj][:],
                            start=True,
                            stop=True,
                        )
                    nc.scalar.activation(
                        e1[:, h * 1024 : (h + 1) * 1024],
                        s1[:],
                        AF.Exp,
                        scale=0.125,
                        accum_out=dh[:, h : h + 1],
                    )
                rr = sp.tile([128, 1], F32, tag="rr", bufs=6, name="rr")
                nc.vector.tensor_add(rr[:], dh[:, 0:1], dh[:, 1:2])
                nc.vector.reciprocal(rr[:], rr[:])
                nc.gpsimd.tensor_scalar_mul(e1[:], e1[:], rr[:])
                nc.sync.dma_start(p_d[b, c * 128 : (c + 1) * 128, :], e1[:])
                yield

        def pass2_tiles(b):
            """Generator: yields after each (half, k-tile) unit."""
            qt_p, kt_p, vn = qts[b], kts[b], vns[b]
            accs = sp.tile([DV, S], F32, tag="accs", name=f"accs{b}")
            accss[b] = accs
            for half in range(2):
                acc = pacc.tile([DV, PW * 2], F32, tag="acc", name=f"acc{b}{half}")
                e2_tiles = [None] * NT

                def emit_pv(t, acc=acc, e2_tiles=e2_tiles):
                    vch = vn[:, t * DV : (t + 1) * DV]
                    for j in range(2):
                        nc.tensor.matmul(
                            acc[:, j * 512 : (j + 1) * 512],
                            vch,
                            e2_tiles[t][:, j * 512 : (j + 1) * 512],
                            start=(t == 0),
                            stop=(t == NT - 1),
                        )

                for t in range(NT):
                    lhs = kt_p[t // 4][:, (t % 4) * 128 : (t % 4 + 1) * 128]
                    e2 = ep.tile([128, PW * 2], F32R, tag="e2", name="e2")
                    e2_tiles[t] = e2
                    s2 = psum.tile([128, 1024], F32, tag="mm", bufs=3, name="s2")
                    for j in range(2):
                        cj = 2 * half + j
                        nc.tensor.matmul(
                            s2[:, j * 512 : (j + 1) * 512],
                            lhs,
                            qt_p[cj][:],
                            start=True,
                            stop=True,
                        )
                    nc.scalar.activation(e2[:], s2[:], AF.Exp, scale=0.125)
                    if t > 0:
                        emit_pv(t - 1)
                    yield
                emit_pv(NT - 1)
                nc.vector.tensor_copy(accs[:, half * 1024 : (half + 1) * 1024], acc[:])

        def finalize_tiles(b):
            accs = accss[b]
            ra = sp.tile([128, NT], F32, tag=f"ra{b}", name=f"ra{b}")
            ob = sp.tile([128, NT * D], F32, tag="ob", name=f"ob{b}")
            for c in range(NT):
                tro = psum.tile([128, DV], F32, tag="mm", bufs=3, name="tro")
                nc.tensor.transpose(
                    tro[:], accs[:, c * 128 : (c + 1) * 128], ident[0:DV, 0:DV]
                )
                nc.vector.reciprocal(ra[:, c : c + 1], tro[:, D : D + 1])
                nc.vector.tensor_scalar_mul(
                    ob[:, c * D : (c + 1) * D], tro[:, 0:D], ra[:, c : c + 1]
                )
                yield
            nc.sync.dma_start(
                out_d[b].rearrange("(c p) d -> p c d", p=128),
                ob[:].rearrange("p (c d) -> p c d", d=D),
            )

        def step(g):
            return next(g, StopIteration) is not StopIteration

        # ---------------- schedule ----------------
        for _ in setup_tiles(0):
            pass
        su1 = setup_tiles(1)
        for b in range(B_LOC):
            g1 = pass1_tiles(b)
            g2 = pass2_tiles(b)
            m1 = m2 = True
            i = 0
            while m1 or m2:
                if m1:
                    m1 = step(g1)
                if m2:
                    m2 = step(g2)
                if m2:
                    m2 = step(g2)
                if b == 0 and i >= 2:
                    step(su1)
                i += 1
            for _ in finalize_tiles(b):
                pass

    nc.compile()
    return nc


_NC = None


def _get_nc():
    global _NC
    if _NC is None:
        _NC = _build()
    return _NC


def kernel(query, key, value, mask, trace=False):
    query = np.ascontiguousarray(np.asarray(query, dtype=np.float32))
    key = np.ascontiguousarray(np.asarray(key, dtype=np.float32))
    value = np.ascontiguousarray(np.asarray(value, dtype=np.float32))
    mask = np.ascontiguousarray(np.asarray(mask, dtype=np.int32))

    nc = _get_nc()
    in_maps = []
    for c in range(N_CORES):
        sl = slice(c * B_LOC, (c + 1) * B_LOC)
        in_maps.append(
            {
                "query": query[sl],
                "key": key[sl],
                "value": value[sl],
                "mask": mask[sl],
            }
        )
    res = run_bass_kernel_spmd(nc, in_maps, core_ids=list(range(N_CORES)), trace=trace)
    out = np.concatenate([res.results[c]["out"] for c in range(N_CORES)], axis=0)
    p_attn = np.concatenate([res.results[c]["p_attn"] for c in range(N_CORES)], axis=0)
    if trace:
        kernel.last_exec_time_ns = res.exec_time_ns
        kernel.last_trace = res.instructions_and_trace
    return out, p_attn


# revision 17
# speedup vs baseline: 4.8473x; 4.8473x over previous
"""Masked attention kernel for Trainium2 (Bass/Tile), 8-core data-parallel.

Full problem: B=16, S=2048, D=64.
  scores = Q @ K^T / 8, masked (mask==0 -> -1e9), p_attn = softmax(scores),
  out = p_attn @ V.  Returns (out, p_attn).

Sharding: batch across 8 cores (2 batches/core), no collectives.

Per-core per-batch plan (float32r matmuls: 1 PE cycle/row at N=512):
  setup:  load Q,K,V natural [128, .]; PE-transpose into 4 column-part tiles
          qt_p/kt_p [65, 512] (row 64: ones / maskbias=(mask-1)*8e9).  The /8
          scale is folded into ACT exp (scale=0.125), the mask into the
          65-row contraction.  V gets a ones column -> vn [128, 16*65].
  pass1:  per q-tile: S[q,k] psum [128,1024]x2 <- qt_tile.T @ kt, E =
          exp(0.125 S) with accum_out -> masked denominators, DVE recip +
          scale -> p_attn tile -> 1MB DMA.
  pass2:  two q-halves; per (half, k-tile): S^T[k,qh] psum [128,1024]
          <- kt_tile.T @ qt (masked, 2 MMs), E^T = exp(0.125 S^T),
          PV: acc[65,1024] += Vext_t.T @ E^T (pipelined one k-tile behind).
          acc row 64 = denominators D[q] for the out normalization.
  final:  acc halves -> accs sbuf; 16 PE transposes [65,128]->[128,65];
          r=1/D (DVE); out chunk = cols 0:64 * r -> one 0.5MB DMA.

Schedule: ACT is the bottleneck (128 exps ~134us) and p_attn DMA second
(~101us).  Per batch the streams are interleaved in trios [1 pass1 tile :
2 pass2 units] so ACT stays dense while the DMA stream never backs up;
batch 1's setup transpose groups are spread into batch 0's trios.
PSUM: "mm" [128,1024] x3 bufs (6 banks) + "acc" [65,1024] x1 (2 banks).
"""

import os
import sys

import numpy as np

for _p in ("/opt/trn_rl_repo", "/root/.axon_site/_ro/trn_rl_repo"):
    if os.path.isdir(_p) and _p not in sys.path:
        sys.path.insert(0, _p)

import concourse.bass as bass
import concourse.mybir as mybir
import concourse.tile as tile
from concourse import bacc
from concourse.bass_utils import run_bass_kernel_spmd
from concourse.masks import make_identity
from contextlib import ExitStack

F32 = mybir.dt.float32
F32R = mybir.dt.float32r
I32 = mybir.dt.int32
AF = mybir.ActivationFunctionType

N_CORES = 8
B_FULL, S, D = 16, 2048, 64
B_LOC = B_FULL // N_CORES  # 2
NT = S // 128  # 16 tiles per seq dim
NP = 4  # column parts of the transposed operands
PW = S // NP  # 512 part width
MASK_SCALE = -8.0e9  # *0.125 -> -1e9
DV = D + 1  # V extended with ones column


def _build():
    nc = bacc.Bacc(
        "TRN2",
        target_bir_lowering=False,
        debug=False,
        enable_asserts=False,
        num_devices=N_CORES,
    )

    q_d = nc.dram_tensor("query", [B_LOC, S, D], F32, kind="ExternalInput").ap()
    k_d = nc.dram_tensor("key", [B_LOC, S, D], F32, kind="ExternalInput").ap()
    v_d = nc.dram_tensor("value", [B_LOC, S, D], F32, kind="ExternalInput").ap()
    m_d = nc.dram_tensor("mask", [B_LOC, 1, S], I32, kind="ExternalInput").ap()
    out_d = nc.dram_tensor("out", [B_LOC, S, D], F32, kind="ExternalOutput").ap()
    p_d = nc.dram_tensor("p_attn", [B_LOC, S, S], F32, kind="ExternalOutput").ap()

    with ExitStack() as ctx:
        tc = ctx.enter_context(tile.TileContext(nc))
        const = ctx.enter_context(tc.tile_pool(name="const", bufs=1))
        nat = ctx.enter_context(tc.tile_pool(name="nat", bufs=1))
        tp = ctx.enter_context(tc.tile_pool(name="tp", bufs=1))
        ep = ctx.enter_context(tc.tile_pool(name="ep", bufs=3))
        sp = ctx.enter_context(tc.tile_pool(name="sp", bufs=1))
        psum = ctx.enter_context(tc.tile_pool(name="psum", bufs=3, space="PSUM"))
        pacc = ctx.enter_context(tc.tile_pool(name="pacc", bufs=1, space="PSUM"))

        ident = const.tile([128, 128], F32, name="ident")
        make_identity(nc, ident[:])
        ones16 = const.tile([128, NT], F32, name="ones16")
        nc.gpsimd.memset(ones16[:], 1.0)

        qts = [[None] * NP for _ in range(B_LOC)]
        kts = [[None] * NP for _ in range(B_LOC)]
        vns = [None] * B_LOC
        accss = [None] * B_LOC

        def setup_tiles(b):
            """Generator: emits loads, then yields after each transpose group."""
            mrow = sp.tile([1, S], F32, tag=f"mrow{b}", name=f"mrow{b}")
            nc.gpsimd.dma_start(mrow[:], m_d[b, 0])

            qn = nat.tile([128, NT * D], F32, tag=f"qn{b}", name=f"qn{b}")
            kn = nat.tile([128, NT * D], F32, tag=f"kn{b}", name=f"kn{b}")
            qv = qn[:].rearrange("p (t d) -> p t d", d=D)
            kv = kn[:].rearrange("p (t d) -> p t d", d=D)
            qs = q_d[b].rearrange("(t p) d -> p t d", p=128)
            ks = k_d[b].rearrange("(t p) d -> p t d", p=128)
            for p in range(NP):
                tsl = slice(4 * p, 4 * p + 4)
                nc.sync.dma_start(qv[:, tsl], qs[:, tsl])
                nc.sync.dma_start(kv[:, tsl], ks[:, tsl])
            vn_raw = nat.tile([128, NT * D], F32, tag=f"vr{b}", name=f"vr{b}")
            nc.sync.dma_start(
                vn_raw[:].rearrange("p (t d) -> p t d", d=D),
                v_d[b].rearrange("(t p) d -> p t d", p=128),
            )
            vn = nat.tile([128, NT * DV], F32R, tag=f"vn{b}", name=f"vn{b}")
            nc.vector.tensor_copy(
                vn[:].rearrange("p (t e) -> p t e", e=DV)[:, :, 0:D],
                vn_raw[:].rearrange("p (t d) -> p t d", d=D),
            )
            nc.vector.tensor_copy(
                vn[:].rearrange("p (t e) -> p t e", e=DV)[:, :, D : D + 1],
                ones16[:].rearrange("p (t u) -> p t u", u=1),
            )
            vns[b] = vn

            def tr_group(src, dst, p):
                tr = psum.tile([D, PW], F32, tag="mm", bufs=3, name="tr")
                for i in range(4):
                    t = 4 * p + i
                    nc.tensor.transpose(
                        tr[:, i * 128 : (i + 1) * 128],
                        src[:, t * D : (t + 1) * D],
                        ident[:],
                    )
                nc.vector.tensor_copy(dst[0:D, :], tr[:])

            def mk_q(p):
                qt = tp.tile([DV, PW], F32R, tag=f"qt{b}{p}", name=f"qt{b}{p}")
                tr_group(qn, qt, p)
                nc.vector.tensor_copy(
                    qt[D : D + 1, :], ones16[0:1, 0:1].broadcast_to([1, PW])
                )
                qts[b][p] = qt

            def mk_k(p):
                kt = tp.tile([DV, PW], F32R, tag=f"kt{b}{p}", name=f"kt{b}{p}")
                tr_group(kn, kt, p)
                nc.vector.tensor_scalar(
                    out=kt[D : D + 1, :],
                    in0=mrow[:, p * PW : (p + 1) * PW],
                    scalar1=1.0,
                    scalar2=-MASK_SCALE,
                    op0=mybir.AluOpType.subtract,
                    op1=mybir.AluOpType.mult,
                )
                kts[b][p] = kt

            # first pass-1 tile needs qt0 + kt0,kt1; first pass-2 unit
            # needs kt0 + qt0,qt1
            for fn, p in (
                (mk_q, 0),
                (mk_k, 0),
                (mk_k, 1),
                (mk_q, 1),
                (mk_q, 2),
                (mk_q, 3),
                (mk_k, 2),
                (mk_k, 3),
            ):
                fn(p)
                yield

        def pass1_tiles(b):
            """Generator: yields after each q-tile unit."""
            qt_p, kt_p = qts[b], kts[b]
            for c in range(NT):
                lhs = qt_p[c // 4][:, (c % 4) * 128 : (c % 4 + 1) * 128]
                e1 = ep.tile([128, S], F32, tag="e1", bufs=6, name="e1")
                dh = sp.tile([128, 2], F32, tag="dh", bufs=6, name="dh")
                for h in range(2):
                    s1 = psum.tile([128, 1024], F32, tag="mm", bufs=3, name="s1")
                    for j in range(2):
                        cj = 2 * h + j
                        nc.tensor.matmul(
                            s1[:, j * 512 : (j + 1) * 512],
                            lhs,
                            kt_p[cj][:],
                            start=True,
                            stop=True,
                        )
                    nc.scalar.activation(
                        e1[:, h * 1024 : (h + 1) * 1024],
                        s1[:],
                        AF.Exp,
                        scale=0.125,
                        accum_out=dh[:, h : h + 1],
                    )
                rr = sp.tile([128, 1], F32, tag="rr", bufs=6, name="rr")
                nc.vector.tensor_add(rr[:], dh[:, 0:1], dh[:, 1:2])
                nc.vector.reciprocal(rr[:], rr[:])
                nc.vector.tensor_scalar_mul(e1[:], e1[:], rr[:])
                nc.sync.dma_start(p_d[b, c * 128 : (c + 1) * 128, :], e1[:])
                yield

        def pass2_tiles(b):
            """Generator: yields after each (half, k-tile) unit."""
            qt_p, kt_p, vn = qts[b], kts[b], vns[b]
            accs = sp.tile([DV, S], F32, tag="accs", name=f"accs{b}")
            accss[b] = accs
            for half in range(2):
                acc = pacc.tile([DV, PW * 2], F32, tag="acc", name=f"acc{b}{half}")
                e2_tiles = [None] * NT

                def emit_pv(t, acc=acc, e2_tiles=e2_tiles):
                    vch = vn[:, t * DV : (t + 1) * DV]
                    for j in range(2):
                        nc.tensor.matmul(
                            acc[:, j * 512 : (j + 1) * 512],
                            vch,
                            e2_tiles[t][:, j * 512 : (j + 1) * 512],
                            start=(t == 0),
                            stop=(t == NT - 1),
                        )

                for t in range(NT):
                    lhs = kt_p[t // 4][:, (t % 4) * 128 : (t % 4 + 1) * 128]
                    e2 = ep.tile([128, PW * 2], F32R, tag="e2", name="e2")
                    e2_tiles[t] = e2
                    s2 = psum.tile([128, 1024], F32, tag="mm", bufs=3, name="s2")
                    for j in range(2):
                        cj = 2 * half + j
                        nc.tensor.matmul(
                            s2[:, j * 512 : (j + 1) * 512],
                            lhs,
                            qt_p[cj][:],
                            start=True,
                            stop=True,
                        )
                    nc.scalar.activation(e2[:], s2[:], AF.Exp, scale=0.125)
                    if t > 0:
                        emit_pv(t - 1)
                    yield
                emit_pv(NT - 1)
                nc.vector.tensor_copy(accs[:, half * 1024 : (half + 1) * 1024], acc[:])

        def finalize_tiles(b):
            accs = accss[b]
            ra = sp.tile([128, NT], F32, tag=f"ra{b}", name=f"ra{b}")
            ob = sp.tile([128, NT * D], F32, tag="ob", name=f"ob{b}")
            for c in range(NT):
                tro = psum.tile([128, DV], F32, tag="mm", bufs=3, name="tro")
                nc.tensor.transpose(
                    tro[:], accs[:, c * 128 : (c + 1) * 128], ident[0:DV, 0:DV]
                )
                nc.vector.reciprocal(ra[:, c : c + 1], tro[:, D : D + 1])
                nc.vector.tensor_scalar_mul(
                    ob[:, c * D : (c + 1) * D], tro[:, 0:D], ra[:, c : c + 1]
                )
                yield
            nc.sync.dma_start(
                out_d[b].rearrange("(c p) d -> p c d", p=128),
                ob[:].rearrange("p (c d) -> p c d", d=D),
            )

        def step(g):
            return next(g, StopIteration) is not StopIteration

        # ---------------- schedule ----------------
        for _ in setup_tiles(0):
            pass
        su1 = setup_tiles(1)
        for b in range(B_LOC):
            g1 = pass1_tiles(b)
            g2 = pass2_tiles(b)
            m1 = m2 = True
            i = 0
            while m1 or m2:
                if m1:
                    m1 = step(g1)
                if m2:
                    m2 = step(g2)
                if m2:
                    m2 = step(g2)
                if b == 0 and i >= 2:
                    step(su1)
                i += 1
            for _ in finalize_tiles(b):
                pass

    nc.compile()
    return nc


_NC = None


def _get_nc():
    global _NC
    if _NC is None:
        _NC = _build()
    return _NC


def kernel(query, key, value, mask, trace=False):
    query = np.ascontiguousarray(np.asarray(query, dtype=np.float32))
    key = np.ascontiguousarray(np.asarray(key, dtype=np.float32))
    value = np.ascontiguousarray(np.asarray(value, dtype=np.float32))
    mask = np.ascontiguousarray(np.asarray(mask, dtype=np.int32))

    nc = _get_nc()
    in_maps = []
    for c in range(N_CORES):
        sl = slice(c * B_LOC, (c + 1) * B_LOC)
        in_maps.append(
            {
                "query": query[sl],
                "key": key[sl],
                "value": value[sl],
                "mask": mask[sl],
            }
        )
    res = run_bass_kernel_spmd(nc, in_maps, core_ids=list(range(N_CORES)), trace=trace)
    out = np.concatenate([res.results[c]["out"] for c in range(N_CORES)], axis=0)
    p_attn = np.concatenate([res.results[c]["p_attn"] for c in range(N_CORES)], axis=0)
    if trace:
        kernel.last_exec_time_ns = res.exec_time_ns
        kernel.last_trace = res.instructions_and_trace
    return out, p_attn
